# revision 40
# baseline (speedup 1.0000x reference)
"""Trainium2 Bass kernel for nn_AE_30142080483951 (gnn_message_passing).

Data-parallel over batch B=8 across 8 NeuronCores (one batch element per
core, weights replicated, no collectives).  Key restructuring vs the
reference:

  - The (M,M) affinity matrix A = SPf^T @ sigma @ SPf is rank-64, so
    A @ yT is computed as SPf^T @ (sigma @ (SPf @ yT)) without ever
    materializing A; the gnn linear is folded into the same low-rank chain.
  - softmax(sp_adj) is data-independent, precomputed on host, scaled by
    1024 into fp8e4m3 (1/1024 folded into sp_w), streamed slice-major
    with the two DoubleRow K-rows in separate column halves.
  - SP is sent PIXEL-MAJOR [p, tile, window, class] so the class-softmax
    sum is a contiguous free-axis vector reduce (no PE colsum matmuls,
    one tiny reciprocal) and the 2x2 maxpool (two pairwise maxes) lands
    directly in token-major layout, feeding the g accumulation without
    any SBUF shuffle.
  - g = SPf @ yT runs as 9 fp8 DoubleRow accumulations; yc/sigma/se stay
    bf16 (fp8 there costs ~4% relative error: random-sign dot products
    keep the full operand quantization error).
  - sigma/hg complete mid-chain so the se tail is not end-gated; the
    3t residual is folded into the back conv as a second accumulation
    pass.
  - DMA queue plan: x alone on the scalar queue (fast kickoff, full
    early bandwidth -> PE starts ~12us), everything else on sync with
    per-slice adjacency triggers so the small early tensors are not
    starved by the 5.3MB stream; outputs ride the gpsimd queue.
  - Engine programs ordered by data arrival: exp chunks interleaved
    with trans relus on scalar, transpose evacuations + softmax chunks
    interleaved on vector; a PE warmup spin ramps the DVFS p-state
    during the input-DMA window.
  - BatchNorms folded to per-channel scale/bias applied by ScalarE
    activations straight out of PSUM (bn1 scale folded into the trans
    weights).  bf16 compute elsewhere (rel tol 2e-2), fp32 PSUM
    accumulation, bf16 output store (f32 cast on host).
"""

import numpy as np
from contextlib import ExitStack

EPS = 1e-5
B, N, Cs, Cin, Ci, Co = 8, 48, 64, 256, 128, 128
M = N * N            # 2304
MT = M // 128        # 18 token tiles
HT2 = MT // 2        # 9 row-pair slices of the adjacency
CH = [(0, 512), (512, 512), (1024, 512), (1536, 512), (2048, 256)]
CHT = [(0, 4), (4, 4), (8, 4), (12, 4), (16, 2)]   # token tiles per chunk
ASP_SCALE = 1024.0   # host-side scale on softmax(sp_adj); folded into sp_w
N_WARM = 40          # PE p-state warmup matmuls

_CACHE = {}


def _build():
    import concourse.bacc as bacc_mod
    import concourse.mybir as mybir
    import concourse.tile as tile
    from concourse.bass import MemorySpace

    f32 = mybir.dt.float32
    bf = mybir.dt.bfloat16
    f8 = mybir.dt.float8e4
    AF = mybir.ActivationFunctionType
    DR = mybir.MatmulPerfMode.DoubleRow
    AX = mybir.AxisListType
    OP = mybir.AluOpType

    nc = bacc_mod.Bacc("TRN2", num_swdge_queues=4)

    # ---- DRAM parameters (per-core shard; bf16/fp8 matmul operands) ----
    x_d = nc.dram_tensor("x", [Cin, M], bf, kind="ExternalInput")
    # pixel-major SP: spt[p, ti*256 + w*64 + c] = SP[c, 2*rp+dy, 2*sp+dx],
    # token m = ti*128+p = rp*48+sp, w = dy*2+dx
    spt_d = nc.dram_tensor("spt", [128, MT * Cs * 4], bf, kind="ExternalInput")
    # host-softmaxed, x1024, fp8 adjacency, split DoubleRow layout:
    # st8[i*128 + p, k*M + m] = softmax(adj).T[256*i + 128*k + p, m] * 1024
    st8_d = nc.dram_tensor("st8", [M // 2, 2 * M], f8, kind="ExternalInput")
    # small first-wave pack (Ci, 384) = [idb(128) | w1t_k0(128) | w1t_k1(128)]
    wia_d = nc.dram_tensor("wia", [Ci, 384], bf, kind="ExternalInput")
    # packed (Ci, 576) = [wkct(64) | gnnwt(128) | spwt(128) | backwt(128) | backwt3(128)]
    wpack_d = nc.dram_tensor("wpack", [Ci, 576], bf, kind="ExternalInput")
    # per-partition-contiguous linNC weight: wnct[p, ti*64+c] = linNC_w.T[ti*128+p, c]
    wnct_d = nc.dram_tensor("wnct", [128, MT * Cs], bf, kind="ExternalInput")
    # packed (Ci, 5) = [bn1b gnnb spb bn2s bn2b]
    bias_d = nc.dram_tensor("biases", [Ci, 5], f32, kind="ExternalInput")
    bkc_d = nc.dram_tensor("bkc", [Cs, 1], f32, kind="ExternalInput")
    bnc_d = nc.dram_tensor("bnc", [1, Cs], bf, kind="ExternalInput")
    gnnbr_d = nc.dram_tensor("gnnbr", [1, Ci], bf, kind="ExternalInput")
    out_d = nc.dram_tensor("out", [Co, M], bf, kind="ExternalOutput")

    tc = tile.TileContext(nc)
    with tc:
        with ExitStack() as ctx:
            ctx.enter_context(
                nc.allow_low_precision(reason="bf16/fp8 compute path, rel tol 2e-2")
            )
            singles = ctx.enter_context(tc.tile_pool(name="singles", bufs=1))
            tails = ctx.enter_context(tc.tile_pool(name="tails", bufs=3))
            psA = ctx.enter_context(
                tc.tile_pool(name="psA", bufs=1, space=MemorySpace.PSUM)
            )
            psS = ctx.enter_context(
                tc.tile_pool(name="psS", bufs=2, space=MemorySpace.PSUM)
            )

            with tc.tile_pool(name="phase1", bufs=1) as p1:
                # ---- DMA plan: x ALONE on the scalar queue (fast kickoff,
                # full early bandwidth); sync FIFO ordered by need time:
                # idb+w1t (gates trans), spT (gates sp path), wnct, wpack,
                # then the 9 adjacency slices ----
                x_sb = p1.tile([128, 2, M], bf)
                nc.scalar.dma_start(
                    out=x_sb[:], in_=x_d[:, :].rearrange("(kt p) m -> p kt m", p=128)
                )
                wia_sb = singles.tile([Ci, 384], bf)
                nc.sync.dma_start(out=wia_sb[:], in_=wia_d[:, :])
                idb = wia_sb[:, 0:128]
                w1t_sb = wia_sb[:, 128:384].rearrange("p (kt c) -> p kt c", kt=2)
                sp_sb = p1.tile([128, MT, 4, Cs], bf)

                def spt_dma(q):
                    nc.sync.dma_start(
                        out=sp_sb[:, 6 * q : 6 * (q + 1), :, :],
                        in_=spt_d[:, 1536 * q : 1536 * (q + 1)].rearrange(
                            "p (t w c) -> p t w c", t=6, w=4
                        ),
                    )

                spt_dma(0)
                wnct_sb = p1.tile([128, MT, Cs], bf)
                nc.sync.dma_start(
                    out=wnct_sb[:],
                    in_=wnct_d[:, :].rearrange("p (t c) -> p t c", t=MT),
                )
                spt_dma(1)
                spt_dma(2)
                wpack_sb = singles.tile([Ci, 576], bf)
                nc.sync.dma_start(out=wpack_sb[:], in_=wpack_d[:, :])
                wkct_sb = wpack_sb[:, 0:64]
                gnnwt_sb = wpack_sb[:, 64:192]
                spwt_sb = wpack_sb[:, 192:320]
                backwt_sb = wpack_sb[:, 320:448]
                backwt3_sb = wpack_sb[:, 448:576]
                est_sb = singles.tile([128, HT2, 2, M], f8)
                for i in range(HT2):
                    nc.sync.dma_start(
                        out=est_sb[:, i, :, :],
                        in_=st8_d[128 * i : 128 * (i + 1), :].rearrange(
                            "p (two m) -> p two m", two=2
                        ),
                    )
                # gpsimd queue: small biases
                bias_sb = singles.tile([Ci, 5], f32)
                nc.gpsimd.dma_start(out=bias_sb[:], in_=bias_d[:, :])
                bn1b_sb = bias_sb[:, 0:1]
                gnnb_sb = bias_sb[:, 1:2]
                spb_sb = bias_sb[:, 2:3]
                bn2s_sb = bias_sb[:, 3:4]
                bn2b_sb = bias_sb[:, 4:5]
                bkc_sb = singles.tile([Cs, 1], f32)
                nc.gpsimd.dma_start(out=bkc_sb[:], in_=bkc_d[:, :])
                bnc_sb = singles.tile([1, Cs], bf)
                nc.gpsimd.dma_start(out=bnc_sb[:], in_=bnc_d[:, :])
                gnnbr_sb = singles.tile([1, Ci], bf)
                nc.gpsimd.dma_start(out=gnnbr_sb[:], in_=gnnbr_d[:, :])

                # persistent activations
                t_sb = singles.tile([Ci, M], bf)
                yT_sb = singles.tile([128, MT, Ci], bf)
                yT8_sb = singles.tile([128, MT, Ci], f8)
                spfT_sb = singles.tile([128, MT, Cs], bf)
                spfT8_sb = singles.tile([128, MT, Cs], f8)
                spfc_sb = singles.tile([Cs, M], bf)
                spre_sb = singles.tile([Ci, M], bf)
                hg_sb = singles.tile([Cs, Ci], bf)

                onesP = p1.tile([128, 128], bf)
                nc.vector.memset(onesP[:], 1.0)
                ones512 = p1.tile([1, 512], bf)
                nc.vector.memset(ones512[:], 1.0)

                # ---- PE warmup spin: ramp DVFS while x streams in ----
                for wi in range(N_WARM):
                    wps = psS.tile([128, 128], f32, tag="ps_small")
                    nc.tensor.matmul(wps[:], onesP[:], onesP[:])

                # 5 PSUM banks shared by trans -> fp8 chain -> tails
                psb = [
                    psA.tile([128, 512], f32, tag=f"ps_chain{j}", name=f"psb{j}")
                    for j in range(len(CH))
                ]

                # ---- t = relu(bn1(W1 @ x)) (bn1 scale folded into w1t) ----
                for j, (mo, mw) in enumerate(CH):
                    nc.tensor.matmul(
                        psb[j][:, :mw],
                        w1t_sb[:, 0, :],
                        x_sb[:, 0, mo : mo + mw],
                        start=True,
                        stop=False,
                    )
                    nc.tensor.matmul(
                        psb[j][:, :mw],
                        w1t_sb[:, 1, :],
                        x_sb[:, 1, mo : mo + mw],
                        start=False,
                        stop=True,
                    )

                spv = sp_sb[:].rearrange("p t w c -> p (t w c)")

                def exp_chunk(q):
                    qs = slice(q * 1536, (q + 1) * 1536)
                    nc.scalar.activation(spv[:, qs], spv[:, qs], AF.Exp)

                def relu_chunk(j):
                    mo, mw = CH[j]
                    nc.scalar.activation(
                        t_sb[:, mo : mo + mw], psb[j][:, :mw], AF.Relu, bias=bn1b_sb
                    )

                # scalar program: first relus (they gate the token
                # transposes -> yT8 -> chain), exp chunks woven between
                relu_chunk(0)
                relu_chunk(1)
                exp_chunk(0)
                relu_chunk(2)
                relu_chunk(3)
                relu_chunk(4)
                exp_chunk(1)
                exp_chunk(2)

                # ---- token transposes of t (PE) + bf16 evacuations and
                # 6-tile fp8 casts (vector) + lagged yc accumulation (PE).
                # The sp softmax path (vector) follows the casts: its
                # products are needed later (g/se), the casts gate the
                # chain. ----
                d_sb = p1.tile([128, MT, 4], f32)
                dinv_sb = p1.tile([128, MT, 4], f32)
                ps_yc = psS.tile([128, Cs], f32, tag="ps_acc", bufs=1)

                def transpose_tile(ti):
                    ps = psS.tile([128, 128], bf, tag="ps_small")
                    nc.tensor.transpose(
                        ps[:], t_sb[:, ti * 128 : (ti + 1) * 128], idb
                    )
                    nc.vector.tensor_copy(yT_sb[:, ti, :], ps[:])
                    if ti > 0:
                        nc.tensor.matmul(
                            ps_yc[:],
                            yT_sb[:, ti - 1, :],
                            wnct_sb[:, ti - 1, :],
                            start=(ti == 1),
                            stop=False,
                        )

                def yt8_cast(q):
                    nc.vector.tensor_copy(
                        yT8_sb[:, 6 * q : 6 * (q + 1), :].rearrange("p t c -> p (t c)"),
                        yT_sb[:, 6 * q : 6 * (q + 1), :].rearrange("p t c -> p (t c)"),
                    )

                for q in range(3):
                    for ti in range(6 * q, 6 * q + 6):
                        transpose_tile(ti)
                    yt8_cast(q)

                nc.tensor.matmul(
                    ps_yc[:],
                    yT_sb[:, MT - 1, :],
                    wnct_sb[:, MT - 1, :],
                    start=False,
                    stop=False,
                )
                nc.tensor.matmul(
                    ps_yc[:], onesP[0:1, :], bnc_sb[:], start=False, stop=True
                )
                yc_sb = p1.tile([Ci, Cs], bf)
                nc.vector.tensor_copy(yc_sb[:], ps_yc[:])

                ps_sg = psS.tile([Cs, Cs], f32, tag="ps_small")
                nc.tensor.matmul(ps_sg[:], wkct_sb, yc_sb[:])
                sigT_sb = p1.tile([Cs, Cs], bf)
                nc.scalar.activation(sigT_sb[:], ps_sg[:], AF.Identity, bias=bkc_sb[:])

                # ---- sp softmax path (vector), pixel-major [p, t, w, c]:
                # contiguous class-sum reduce, tiny reciprocal, broadcast
                # normalize, two pairwise maxes -> token-major spfT ----
                for q in range(3):
                    ts_ = slice(q * 6, (q + 1) * 6)
                    nc.vector.tensor_reduce(
                        out=d_sb[:, ts_, :],
                        in_=sp_sb[:, ts_, :, :],
                        axis=AX.X,
                        op=OP.add,
                    )
                    nc.vector.reciprocal_approx_fast(
                        dinv_sb[:, ts_, :].rearrange("p t w -> p (t w)"),
                        d_sb[:, ts_, :].rearrange("p t w -> p (t w)"),
                    )
                    nc.vector.tensor_tensor(
                        out=sp_sb[:, ts_, :, :],
                        in0=sp_sb[:, ts_, :, :],
                        in1=dinv_sb[:, ts_, :]
                        .rearrange("p t (w one) -> p t w one", one=1)
                        .broadcast_to([128, 6, 4, Cs]),
                        op=OP.mult,
                    )
                    nc.vector.tensor_tensor(
                        out=sp_sb[:, ts_, 0:2, :],
                        in0=sp_sb[:, ts_, 0:2, :],
                        in1=sp_sb[:, ts_, 2:4, :],
                        op=OP.max,
                    )
                    nc.vector.tensor_tensor(
                        out=spfT_sb[:, ts_, :],
                        in0=sp_sb[:, ts_, 0:1, :].rearrange("p t one c -> p t (one c)"),
                        in1=sp_sb[:, ts_, 1:2, :].rearrange("p t one c -> p t (one c)"),
                        op=OP.max,
                    )
                # fp8 shadow for the g DoubleRow accumulation
                nc.vector.tensor_copy(
                    spfT8_sb[:].rearrange("p t c -> p (t c)"),
                    spfT_sb[:].rearrange("p t c -> p (t c)"),
                )

                # ---- fp8 DoubleRow chain, i-major: one LDWEIGHTS per
                # row-pair slice serves all 5 PSUM banks ----
                for i in range(HT2):
                    for j, (mo, mw) in enumerate(CH):
                        nc.tensor.matmul(
                            psb[j][:, :mw],
                            yT8_sb[:, 2 * i : 2 * i + 2, :],
                            est_sb[:, i, :, mo : mo + mw],
                            start=(i == 0),
                            stop=(i == HT2 - 1),
                            perf_mode=DR,
                        )

                # ---- g (cs, ci) via 9 fp8 DoubleRow accumulations, then
                # ht/hg for the se branch ----
                ps_g = psS.tile([Cs, Ci], f32, tag="ps_acc", bufs=1)
                for i in range(HT2):
                    nc.tensor.matmul(
                        ps_g[:],
                        spfT8_sb[:, 2 * i : 2 * i + 2, :],
                        yT8_sb[:, 2 * i : 2 * i + 2, :],
                        start=(i == 0),
                        stop=(i == HT2 - 1),
                        perf_mode=DR,
                    )
                g_sb = p1.tile([Cs, Ci], bf)
                nc.vector.tensor_copy(g_sb[:], ps_g[:])

                ps_ht = psS.tile([Ci, Cs], f32, tag="ps_small")
                nc.tensor.matmul(ps_ht[:], g_sb[:], sigT_sb[:])
                ht_sb = p1.tile([Ci, Cs], bf)
                nc.vector.tensor_copy(ht_sb[:], ps_ht[:])

                ps_hg = psS.tile([Cs, Ci], f32, tag="ps_small")
                nc.tensor.matmul(ps_hg[:], ht_sb[:], gnnwt_sb)
                nc.vector.tensor_copy(hg_sb[:], ps_hg[:])

                # ---- tails: the se matmuls run in the spare ps_acc
                # bank in parallel with the sp-linear chain on the main
                # banks; y3a/rse bias-relu on vector, spre/ob on scalar ----
                def spf_transpose(ti):
                    ps = psS.tile([Cs, 128], bf, tag="ps_small")
                    nc.tensor.transpose(ps[:], spfT_sb[:, ti, :], idb)
                    if ti % 2 == 0:
                        nc.scalar.activation(
                            spfc_sb[:, ti * 128 : (ti + 1) * 128], ps[:], AF.Copy
                        )
                    else:
                        nc.vector.tensor_copy(
                            spfc_sb[:, ti * 128 : (ti + 1) * 128], ps[:]
                        )

                def se_branch(j):
                    # se psum accumulates hg@spfc + the rank-1 gnn bias, so
                    # the tail fuses relu(se)+y3a into ONE vector op
                    mo, mw = CH[j]
                    sl_ = slice(mo, mo + mw)
                    pse = psS.tile([128, 512], f32, tag="ps_acc", bufs=1)
                    nc.tensor.matmul(
                        pse[:, :mw], hg_sb[:], spfc_sb[:, sl_], start=True, stop=False
                    )
                    nc.tensor.matmul(
                        pse[:, :mw], gnnbr_sb[:], ones512[:, :mw],
                        start=False, stop=True,
                    )
                    return pse

                def tail_a(j):
                    mo, mw = CH[j]
                    sl_ = slice(mo, mo + mw)
                    # spre holds 1024*(Asp @ yT); 1/1024 folded into spwt
                    nc.scalar.activation(spre_sb[:, sl_], psb[j][:, :mw], AF.Copy)
                    nc.tensor.matmul(psb[j][:, :mw], spwt_sb, spre_sb[:, sl_])
                    y3a = tails.tile([128, 512], bf, tag="y3a", bufs=5)
                    nc.vector.tensor_scalar(
                        out=y3a[:, :mw],
                        in0=psb[j][:, :mw],
                        scalar1=spb_sb,
                        scalar2=0.0,
                        op0=OP.add,
                        op1=OP.max,
                    )
                    return y3a

                def tail_b(j, y3a, pse):
                    mo, mw = CH[j]
                    sl_ = slice(mo, mo + mw)
                    y3b = tails.tile([128, 512], bf, tag="y3b")
                    nc.vector.scalar_tensor_tensor(
                        out=y3b[:, :mw],
                        in0=pse[:, :mw],
                        scalar=0.0,
                        in1=y3a[:, :mw],
                        op0=OP.max,
                        op1=OP.add,
                    )
                    nc.tensor.matmul(
                        psb[j][:, :mw], backwt_sb, y3b[:, :mw], start=True, stop=False
                    )
                    nc.tensor.matmul(
                        psb[j][:, :mw], backwt3_sb, t_sb[:, sl_], start=False, stop=True
                    )
                    ob = tails.tile([128, 512], bf, tag="ob")
                    nc.scalar.activation(
                        ob[:, :mw],
                        psb[j][:, :mw],
                        AF.Relu,
                        bias=bn2b_sb,
                        scale=bn2s_sb,
                    )
                    nc.sync.dma_start(out=out_d[:, sl_], in_=ob[:, :mw])

                y3as = []
                for j in range(len(CH)):
                    for ti in range(CHT[j][0], CHT[j][0] + CHT[j][1]):
                        spf_transpose(ti)
                    y3as.append((tail_a(j), se_branch(j)))
                for j in range(len(CH)):
                    tail_b(j, y3as[j][0], y3as[j][1])

    nc.finalize()
    return nc


def _host_prep(inputs):
    """Fold BNs, transpose weights, precompute softmax(sp_adj) (parameter-
    only), cast matmul operands to bf16/fp8, build the 8 per-core input
    maps (core b gets batch element b)."""
    import ml_dtypes

    f = np.float32
    bf = ml_dtypes.bfloat16
    f8 = ml_dtypes.float8_e4m3
    x = np.ascontiguousarray(inputs["x"], dtype=f).reshape(B, Cin, M)
    SP = np.ascontiguousarray(inputs["SP"], dtype=f)  # (B, Cs, 96, 96)

    bn1s = (np.asarray(inputs["bn1_gamma"]) / np.sqrt(np.asarray(inputs["bn1_var"]) + EPS)).astype(f)
    bn1b = (np.asarray(inputs["bn1_beta"]) - np.asarray(inputs["bn1_mean"]) * bn1s).astype(f)
    bn2s = (np.asarray(inputs["bn2_gamma"]) / np.sqrt(np.asarray(inputs["bn2_var"]) + EPS)).astype(f)
    bn2b = (np.asarray(inputs["bn2_beta"]) - np.asarray(inputs["bn2_mean"]) * bn2s).astype(f)

    # softmax over the last axis of the learned adjacency; split DoubleRow
    # fp8 layout: st8[i*128+p, k*M+m] = AspT[256i+128k+p, m]
    adj = np.asarray(inputs["sp_adj"], dtype=np.float64)
    e = np.exp(adj - adj.max(axis=1, keepdims=True))
    asp = e / e.sum(axis=1, keepdims=True)
    aspT = (asp.T * ASP_SCALE).astype(f)                      # (M, M)
    spl = aspT.reshape(HT2, 2, 128, M).transpose(0, 2, 1, 3)  # (i, p, k, m)
    st8 = np.ascontiguousarray(spl.reshape(M // 2, 2 * M)).astype(f8)

    # bn1 scale folded into trans weight
    w1t = (np.asarray(inputs["trans_w"]).T * bn1s[None, :]).astype(f)  # (Cin, Ci)

    wia = np.concatenate(
        [
            np.eye(128, dtype=f),                               # idb (128, 128)
            w1t[0:128, :],                                      # w1t k-rows 0-127
            w1t[128:256, :],                                    # w1t k-rows 128-255
        ],
        axis=1,
    ).astype(bf)
    wpack = np.concatenate(
        [
            np.asarray(inputs["linKC_w"]).T,                    # (128, 64)
            np.asarray(inputs["gnn_w"]).T,                      # (128, 128)
            np.asarray(inputs["sp_w"]).T / ASP_SCALE,           # (128, 128)
            np.asarray(inputs["back_w"]).T,                     # (128, 128)
            np.asarray(inputs["back_w"]).T * 3.0,               # (128, 128)
        ],
        axis=1,
    ).astype(bf)

    biases = np.stack([bn1b,
                       np.asarray(inputs["gnn_b"], dtype=f),
                       np.asarray(inputs["sp_b"], dtype=f),
                       bn2s, bn2b], axis=1).astype(f)

    # per-partition contiguous linNC weight: wnct[p, ti*64+c]
    wnct = np.asarray(inputs["linNC_w"]).T.reshape(MT, 128, Cs)
    wnct = np.ascontiguousarray(wnct.transpose(1, 0, 2)).reshape(128, MT * Cs)

    # pixel-major SP layout: [p, ti, w, c]
    spt = SP.reshape(B, Cs, 48, 2, 48, 2).transpose(0, 2, 4, 3, 5, 1)
    spt = spt.reshape(B, M, 4 * Cs).reshape(B, MT, 128, 4 * Cs)
    spt = np.ascontiguousarray(spt.transpose(0, 2, 1, 3)).reshape(B, 128, MT * Cs * 4)

    shared = {
        "st8": st8,
        "wia": np.ascontiguousarray(wia),
        "wpack": np.ascontiguousarray(wpack),
        "wnct": np.ascontiguousarray(wnct).astype(bf),
        "biases": np.ascontiguousarray(biases),
        "bkc": np.asarray(inputs["linKC_b"], dtype=f).reshape(Cs, 1),
        "bnc": np.asarray(inputs["linNC_b"], dtype=f).reshape(1, Cs).astype(bf),
        "gnnbr": np.asarray(inputs["gnn_b"], dtype=f).reshape(1, Ci).astype(bf),
    }
    in_maps = []
    for b in range(B):
        m = dict(shared)
        m["x"] = np.ascontiguousarray(x[b]).astype(bf)
        m["spt"] = np.ascontiguousarray(spt[b]).astype(bf)
        in_maps.append(m)
    return in_maps


def _get_nc():
    if "nc" not in _CACHE:
        _CACHE["nc"] = _build()
    return _CACHE["nc"]


def run_spmd(inputs, trace=False, trace_cores=None):
    """Build (cached), run on cores 0-7, return BassKernelResults."""
    from concourse.bass_utils import run_bass_kernel_spmd

    nc = _get_nc()
    in_maps = _host_prep(inputs)
    kwargs = {}
    if trace:
        kwargs = dict(trace=True, trace_cores=trace_cores or [0])
    return run_bass_kernel_spmd(nc, in_maps, core_ids=list(range(8)), **kwargs)


def kernel(**inputs):
    res = run_spmd(inputs)
    out = np.stack([r["out"].reshape(Co, N, N) for r in res.results])
    return out.astype(np.float32)
